# revision 5
# baseline (speedup 1.0000x reference)
"""Trainium2 Bass kernel for nn_AttentionBlock — fp8e4 DoubleRow edition.

Math (equivalent to the reference up to fp rounding):
  - GroupNorm folded to per-channel scale/offset. Per-channel (mean, E[x^2])
    via DVE bn_stats/bn_aggr (one pass, no ACT work); group reduce via a
    bf16 block-diagonal matmul; rstd = 1/sqrt(var+eps) by two DVE Newton
    steps from r0=1 (var is 1 +- ~6% here). ACT thus only ever runs
    {Identity, Exp}: one activation-table load for the whole kernel.
  - Scores computed as hn^T (wq^T wk) hn with M = wq^T wk precomputed on
    the host: q and k never materialize. The q-bias term becomes a per-m
    additive bias u^T hn (u = wk^T bq), computed by tiny N=1 matmuls that
    share the v-matmul's LDWEIGHTS, and fed to Exp's per-partition bias.
  - k bias dropped (constant along softmax axis); v bias folded into the
    proj bias pb = proj_w @ bv + proj_b, which rides the host-precomputed
    bf16 residual x+pb, so the epilogue is one DVE op per tile.
  - The x fed to stats/hn and the residual are bf16 (halves the
    HBM-contended startup DMA and the output traffic; rel err 2.5e-3 vs
    the 2e-2 gate; the attention path is fp8 anyway).
  - Softmax denominator: ones(1/16)-matmul accumulated over pT on the PE
    -> rbc = 16/den; o lands 16x scaled, proj weights are 16x scaled, the
    epilogue applies 1/256. All fp8 tensors sit in e4m3's normal range.
  - All big matmuls are fp8e4 DoubleRow: 2 weights/PE cell, K=256 per
    instruction, 2x bf16 FLOP rate.

Scheduling notes:
  - v, ub and both score halves for one token block share one LDWEIGHTS
    (lhsT = the same hn chunk pair); a pre-compile pass drops the
    redundant InstLdweights that bass emits per matmul (walrus does not
    dedup identical consecutive weight loads itself).
  - Softmax-denominator + attn@v of sample s interleave with sample s+1's
    t/v/scores matmuls; sample 1's scores interleave with sample 0's proj.
  - Junk DoubleRow matmuls with no DMA deps run at startup so the PE HAM
    clock-gate is warm when the real stream begins.

Measured numpy simulation of this exact quantization layout: rel err
7.7e-4 vs the fp32 reference (gate is 2e-2; the output is dominated by
the fp32 residual x, diluting attention-path noise ~30x).
"""

import math
import numpy as np

import concourse.bass as bass
import concourse.bacc as bacc
import concourse.tile as tile
from concourse import bass_isa, mybir
from concourse.bass_utils import run_bass_kernel_spmd

F32 = mybir.dt.float32
F8 = mybir.dt.float8e4
AF = mybir.ActivationFunctionType
OP = mybir.AluOpType
AX = mybir.AxisListType
DR = mybir.MatmulPerfMode.DoubleRow

B = 16
C = 512
HW = 1024
NCORES = 8
SPC = B // NCORES          # samples per core
KO = C // 128              # channel chunks of 128
KJ = KO // 2               # DoubleRow channel-chunk pairs
MI = HW // 128             # token chunks of 128
MJ = MI // 2               # DoubleRow token-chunk pairs
NH = HW // 512             # 512-wide column halves
EPS = 1e-5
SM_SCALE = 1.0 / math.sqrt(C)
N_WARM = 26
N_WARM2 = 0
N_WARM3 = 12
BF16 = mybir.dt.bfloat16


def dedup_ldweights(nc) -> int:
    """Drop InstLdweights that reload the exact weights already resident.

    bass emits one InstLdweights per matmul; for runs of matmuls sharing
    the same stationary operand only the first load is needed. Safe iff
    between the kept load and the candidate there are only PE matmuls
    (any other PE instruction — waits, drains — may order a write to the
    weights region, so it resets the window) and the candidate introduces
    no dependency edges beyond the kept load's.
    """
    removed = 0
    for blk in nc.main_func.blocks:
        insts = blk.instructions
        keep = []
        last_sig = None
        last_deps: tuple[set, set] | None = None
        between: set[str] = set()
        for i in insts:
            if isinstance(i, mybir.InstLdweights):
                sig = (str(i.ins[0]), str(getattr(i, "perf_mode", None)),
                       str(getattr(i, "tile_position", None)))
                sd = set(i.sync_dependency_names())
                nd = set(i.nosync_dependency_names())
                if (last_sig == sig and last_deps is not None
                        and sd <= last_deps[0]
                        and nd <= (last_deps[1] | between)):
                    removed += 1
                    continue
                last_sig = sig
                last_deps = (sd, nd)
                between = set()
            elif isinstance(i, mybir.InstMatmult):
                between.add(i.name)
            elif i.engine == mybir.EngineType.PE:
                last_sig = None
                last_deps = None
            keep.append(i)
        if removed:
            del insts[:]
            insts.extend(keep)
    return removed


def build() -> bass.Bass:
    nc = bacc.Bacc()

    xb_h = nc.declare_dram_parameter("xb", [SPC, C, HW], BF16, isOutput=False)
    xpb_h = nc.declare_dram_parameter("xpb", [SPC, C, HW], BF16, isOutput=False)
    m_h = nc.declare_dram_parameter("m16", [C, C], F8, isOutput=False)
    wv_h = nc.declare_dram_parameter("wv16", [C, C], F8, isOutput=False)
    wp_h = nc.declare_dram_parameter("wp16", [C, C], F8, isOutput=False)
    u_h = nc.declare_dram_parameter("u16", [C, 16], F8, isOutput=False)
    pb_h = nc.declare_dram_parameter("pb", [C], F32, isOutput=False)
    gam_h = nc.declare_dram_parameter("gam", [C], F32, isOutput=False)
    bet_h = nc.declare_dram_parameter("bet", [C], F32, isOutput=False)
    gs_h = nc.declare_dram_parameter("gs", [128, 128], mybir.dt.bfloat16, isOutput=False)
    on_h = nc.declare_dram_parameter("on16", [128, 256], F8, isOutput=False)
    y_h = nc.declare_dram_parameter("y", [SPC, C, HW], BF16, isOutput=True)

    with tile.TileContext(nc) as tc:
        with (
            tc.tile_pool(name="const", bufs=1) as const,
            tc.tile_pool(name="xp", bufs=2) as xp,
            tc.tile_pool(name="work", bufs=1) as work,
            tc.tile_pool(name="hnp", bufs=2) as hnp,
            tc.tile_pool(name="small", bufs=2) as small,
            tc.tile_pool(name="yp", bufs=3) as yp,
            tc.tile_pool(name="ps_mm", bufs=3, space="PSUM") as ps_mm,
            tc.tile_pool(name="ps_big", bufs=2, space="PSUM") as ps_big,
            tc.tile_pool(name="ps_ub", bufs=1, space="PSUM") as ps_ub,
        ):
            # ---- PE warmup (HAM un-throttle). Junk MMs go to the ub
            # pool's bank: its tiles have no readers, so the junk stream
            # never waits on evacuations and can fill any dependency gap.
            junkW = const.tile([128, 2, 128], F8, tag="junkW")
            nc.vector.memset(junkW, 0.0)
            junkR = const.tile([128, 2, 512], F8, tag="junkR")
            nc.vector.memset(junkR, 0.0)
            warm_n = [0]

            def emit_warmup(n):
                for _ in range(n):
                    wps = ps_mm.tile([128, 512], F32, tag="mm",
                                     name=f"warm_{warm_n[0]}")
                    warm_n[0] += 1
                    nc.tensor.matmul(wps, lhsT=junkW, rhs=junkR,
                                     start=True, stop=True, perf_mode=DR,
                                     skip_group_check=True)

            emit_warmup(N_WARM)

            # ---- input DMAs. The stats/hn path reads a bf16 copy of x
            # (half the bytes of the HBM-contended startup burst); the fp32
            # x for the residual streams later, it is first read ~40us in.
            xb_sbs = [[xp.tile([128, HW], BF16, tag=f"xb{ko}", name=f"xb_sb_{s}_{ko}")
                       for ko in range(KO)] for s in range(SPC)]
            # half-chunk transfers so bn_stats can chase the DMA stream;
            # halves split across two DGE queues (sync + idle gpsimd)
            for ko in range(KO):
                for h in range(2):
                    eng = nc.sync if h == 0 else nc.gpsimd
                    eng.dma_start(
                        out=xb_sbs[0][ko][:, h * 512:(h + 1) * 512],
                        in_=xb_h[0][ko * 128:(ko + 1) * 128, h * 512:(h + 1) * 512])
            gs_sb = const.tile([128, 128], mybir.dt.bfloat16, tag="gs")
            nc.sync.dma_start(out=gs_sb, in_=gs_h[:])
            m_sb = const.tile([128, KO, C], F8, tag="m16")
            nc.sync.dma_start(out=m_sb, in_=m_h[:].rearrange("(ki p) n -> p ki n", p=128))
            wv_sb = const.tile([128, KO, C], F8, tag="wv")
            nc.sync.dma_start(out=wv_sb, in_=wv_h[:].rearrange("(ki p) n -> p ki n", p=128))
            u_sb = const.tile([128, KO, 16], F8, tag="u")
            nc.sync.dma_start(out=u_sb, in_=u_h[:].rearrange("(ki p) z -> p ki z", p=128))
            pb_sb = const.tile([128, KO], F32, tag="pb")
            nc.sync.dma_start(out=pb_sb, in_=pb_h[:].rearrange("(ko p) -> p ko", p=128))
            gam_sb = const.tile([128, KO], F32, tag="gam")
            nc.sync.dma_start(out=gam_sb, in_=gam_h[:].rearrange("(ko p) -> p ko", p=128))
            bet_sb = const.tile([128, KO], F32, tag="bet")
            nc.sync.dma_start(out=bet_sb, in_=bet_h[:].rearrange("(ko p) -> p ko", p=128))
            for ko in range(KO):
                nc.sync.dma_start(out=xb_sbs[1][ko],
                                  in_=xb_h[1][ko * 128:(ko + 1) * 128, :])
            on_sb = const.tile([128, 2, 128], F8, tag="on")
            nc.sync.dma_start(out=on_sb, in_=on_h[:].rearrange("p (t k) -> p t k", t=2))
            wp_sb = const.tile([128, KO, C], F8, tag="wp")
            nc.gpsimd.dma_start(out=wp_sb, in_=wp_h[:].rearrange("(ki p) n -> p ki n", p=128))
            # residual-with-bias copies, first read by the proj epilogues
            xpb_sbs = [[xp.tile([128, HW], BF16, tag=f"xpb{ko}", name=f"xpb_sb_{s}_{ko}")
                        for ko in range(KO)] for s in range(SPC)]
            for s in range(SPC):
                for ko in range(KO):
                    nc.sync.dma_start(out=xpb_sbs[s][ko],
                                      in_=xpb_h[s][ko * 128:(ko + 1) * 128, :])

            eps_sb = const.tile([128, 1], F32, tag="eps")
            nc.vector.memset(eps_sb, EPS)
            i16_sb = const.tile([128, 1], F32, tag="i16")
            nc.vector.memset(i16_sb, 1.0 / 16.0)
            smsc_sb = const.tile([128, 1], F32, tag="smsc")
            nc.vector.memset(smsc_sb, SM_SCALE / 16.0)
            i256_sb = const.tile([128, 1], F32, tag="i256")
            nc.vector.memset(i256_sb, 1.0 / 256.0)
            zero_sb = const.tile([128, 1], F32, tag="zero")
            nc.vector.memset(zero_sb, 0.0)
            junk_f = const.tile([128, HW], F32, tag="junkF")
            m05_sb = const.tile([128, 1], F32, tag="m05")
            nc.vector.memset(m05_sb, -0.5)
            c15_sb = const.tile([128, 1], F32, tag="c15")
            nc.vector.memset(c15_sb, 1.5)

            def emit_stats(s, act_chunks=()):
                """(mean, E[x^2]) per channel: DVE bn_stats one-pass; the
                chunks in act_chunks instead use ACT accumulation with the
                scale trick (Identity(x/1024) sums to the mean, Square(x/32)
                sums to E[x^2]) to offload the DVE where it is congested.
                st in bf16 so the group matmul runs single-pass."""
                bn = small.tile([128, KO, 2, 6], F32, tag="bn", name=f"bn_{s}")
                st = small.tile([128, KO, 2], mybir.dt.bfloat16, tag="st",
                                name=f"st_{s}")
                for ko in range(KO):
                    if ko in act_chunks:
                        # ACT's accumulator is fp32 internally; only the
                        # final store rounds to bf16
                        with nc.allow_low_precision(reason="fp32 accum, bf16 store"):
                            nc.scalar.activation(out=junk_f, in_=xb_sbs[s][ko],
                                                 func=AF.Identity, bias=zero_sb,
                                                 scale=1.0 / 1024.0,
                                                 accum_out=st[:, ko, 0:1])
                            nc.scalar.activation(out=junk_f, in_=xb_sbs[s][ko],
                                                 func=AF.Square, bias=zero_sb,
                                                 scale=1.0 / 32.0,
                                                 accum_out=st[:, ko, 1:2])
                        continue
                    for h in range(2):
                        nc.vector.bn_stats(out=bn[:, ko, h, :],
                                           in_=xb_sbs[s][ko][:, h * 512:(h + 1) * 512])
                    nc.vector.bn_aggr(out=st[:, ko, :], in_=bn[:, ko, :, :])
                # var -> E[x^2] fixup, only for the bn chunks
                msq = small.tile([128, KO], F32, tag="msq", name=f"msq_{s}")
                for ko in range(KO):
                    if ko in act_chunks:
                        continue
                    nc.vector.tensor_mul(msq[:, ko:ko + 1], st[:, ko, 0:1],
                                         st[:, ko, 0:1])
                    nc.vector.tensor_add(st[:, ko, 1:2], st[:, ko, 1:2],
                                         msq[:, ko:ko + 1])
                return st

            def emit_scloff(s, st):
                gpt = ps_mm.tile([128, 512], F32, tag="mm", name=f"gps_{s}")
                for ko in range(KO):
                    nc.tensor.matmul(gpt[:, 2 * ko:2 * ko + 2], lhsT=gs_sb,
                                     rhs=st[:, ko, :], start=True, stop=True)
                gps = gpt[:, 0:8].rearrange("p (ko t) -> p ko t", t=2)
                mean = small.tile([128, KO], F32, tag="mean", name=f"mean_{s}")
                nc.vector.tensor_copy(out=mean, in_=gps[:, :, 0])
                var = small.tile([128, KO], F32, tag="var", name=f"var_{s}")
                nc.vector.tensor_mul(var, mean, mean)
                nc.vector.tensor_sub(var, gps[:, :, 1], var)
                # rstd = 1/sqrt(var+eps) via two Newton steps from r0=1 on
                # DVE (group var over 16k unit-normal samples is 1 +- ~6%,
                # so convergence is ~1e-5): keeps Sqrt off ACT, which then
                # needs exactly one activation-table set for the kernel.
                vpe = small.tile([128, KO], F32, tag="vpe", name=f"vpe_{s}")
                nc.vector.tensor_scalar_add(out=vpe, in0=var, scalar1=eps_sb)
                rstd = small.tile([128, KO], F32, tag="rstd", name=f"rstd_{s}")
                nc.vector.tensor_scalar(out=rstd, in0=vpe, scalar1=m05_sb,
                                        scalar2=c15_sb, op0=OP.mult, op1=OP.add)
                rr = small.tile([128, KO], F32, tag="rr", name=f"rr_{s}")
                nc.vector.tensor_mul(rr, rstd, rstd)
                nc.vector.tensor_mul(rr, vpe, rr)
                nc.vector.tensor_scalar(out=rr, in0=rr, scalar1=m05_sb,
                                        scalar2=c15_sb, op0=OP.mult, op1=OP.add)
                nc.vector.tensor_mul(rstd, rstd, rr)
                scl = small.tile([128, KO], F32, tag="scl", name=f"scl_{s}")
                nc.vector.tensor_mul(scl, rstd, gam_sb)
                off = small.tile([128, KO], F32, tag="off", name=f"off_{s}")
                nc.vector.tensor_mul(off, mean, scl)
                nc.vector.tensor_sub(off, bet_sb, off)
                return scl, off

            def emit_hn(s, scl, off):
                hn = hnp.tile([128, KO, HW], F8, tag="hn", name=f"hn_{s}")
                for ko in range(KO):
                    if ko in (1, 2) or s == 1:
                        nc.scalar.activation(
                            out=hn[:, ko, :], in_=xb_sbs[s][ko],
                            func=AF.Identity, bias=off[:, ko:ko + 1],
                            scale=scl[:, ko:ko + 1])
                    else:
                        nc.vector.tensor_scalar(
                            out=hn[:, ko, :], in0=xb_sbs[s][ko],
                            scalar1=scl[:, ko:ko + 1], scalar2=off[:, ko:ko + 1],
                            op0=OP.mult, op1=OP.add)
                return hn

            def t_steps(s, hn):
                """t = M^T hn, channel-major fp8 (ACT evacuates)."""
                t8 = work.tile([128, KO, HW], F8, tag="t", name=f"t_{s}")

                def t_step(mo):
                    def go():
                        tp = [ps_mm.tile([128, 512], F32, tag="mm",
                                         name=f"t_{s}_{mo}_{nh}") for nh in range(NH)]
                        for kj in range(KJ):
                            for nh in range(NH):
                                nc.tensor.matmul(
                                    tp[nh],
                                    lhsT=m_sb[:, 2 * kj:2 * kj + 2, mo * 128:(mo + 1) * 128],
                                    rhs=hn[:, 2 * kj:2 * kj + 2, nh * 512:(nh + 1) * 512],
                                    start=(kj == 0), stop=(kj == KJ - 1), perf_mode=DR)
                        # evacuations: split ACT/DVE for sample 0 (its t
                        # phase is latency-critical); all-ACT for sample 1
                        # (hidden under denav0, where the DVE is the busy one)
                        nc.scalar.activation(
                            out=t8[:, mo, 0:512], in_=tp[0],
                            func=AF.Identity, bias=eps_sb, scale=1.0 / 16.0)
                        if s == 0:
                            nc.vector.tensor_scalar_mul(
                                out=t8[:, mo, 512:1024], in0=tp[1], scalar1=i16_sb)
                        else:
                            nc.scalar.activation(
                                out=t8[:, mo, 512:1024], in_=tp[1],
                                func=AF.Identity, bias=eps_sb, scale=1.0 / 16.0)
                    return go
                return [t_step(mo) for mo in range(KO)], t8

            def vsc_steps(s, hn, t8):
                """Fused v / ub / scores / exp per token block: all four
                matmul kinds share one LDWEIGHTS of the hn chunk pair."""
                v8 = work.tile([128, MI, C], F8, tag="v", name=f"v_{s}")
                pT = work.tile([128, MI, HW], F8, tag="pT", name=f"pT_{s}")
                ub_sb = small.tile([128, MI], F32, tag="ubs", name=f"ubs_{s}")

                def step(mi):
                    def go():
                        vp = ps_mm.tile([128, 512], F32, tag="mm", name=f"v_{s}_{mi}")
                        up = ps_ub.tile([128, 512], F32, tag="ub", name=f"ub_{s}_{mi}")
                        sp = ps_big.tile([128, HW], F32, tag="big", name=f"sc_{s}_{mi}")
                        for kj in range(KJ):
                            lhsT = hn[:, 2 * kj:2 * kj + 2, mi * 128:(mi + 1) * 128]
                            st, sp_ = (kj == 0), (kj == KJ - 1)
                            nc.tensor.matmul(vp, lhsT=lhsT,
                                             rhs=wv_sb[:, 2 * kj:2 * kj + 2, :],
                                             start=st, stop=sp_, perf_mode=DR)
                            nc.tensor.matmul(up[:, 0:1], lhsT=lhsT,
                                             rhs=u_sb[:, 2 * kj:2 * kj + 2, 0:1],
                                             start=st, stop=sp_, perf_mode=DR)
                            for nh in range(NH):
                                nc.tensor.matmul(
                                    sp[:, nh * 512:(nh + 1) * 512], lhsT=lhsT,
                                    rhs=t8[:, 2 * kj:2 * kj + 2, nh * 512:(nh + 1) * 512],
                                    start=st, stop=sp_, perf_mode=DR)
                        if mi % 2 == 0:
                            nc.vector.tensor_scalar_mul(out=v8[:, mi, :], in0=vp,
                                                        scalar1=i16_sb)
                        else:
                            nc.scalar.activation(out=v8[:, mi, :], in_=vp,
                                                 func=AF.Identity, bias=eps_sb,
                                                 scale=1.0 / 16.0)
                        nc.vector.tensor_scalar_mul(out=ub_sb[:, mi:mi + 1],
                                                    in0=up[:, 0:1], scalar1=smsc_sb)
                        nc.scalar.activation(out=pT[:, mi, :], in_=sp,
                                             func=AF.Exp, bias=ub_sb[:, mi:mi + 1],
                                             scale=SM_SCALE)
                    return go
                return [step(mi) for mi in range(MI)], v8, pT

            def denav_steps(s, v8, pT, o8):
                rbc = work.tile([128, HW], F32, tag="rbc", name=f"rbc_{s}")

                def den_step():
                    dp = ps_big.tile([128, HW], F32, tag="big", name=f"den_{s}")
                    for nh in range(NH):
                        for mj in range(MJ):
                            nc.tensor.matmul(
                                dp[:, nh * 512:(nh + 1) * 512], lhsT=on_sb,
                                rhs=pT[:, 2 * mj:2 * mj + 2, nh * 512:(nh + 1) * 512],
                                start=(mj == 0), stop=(mj == MJ - 1), perf_mode=DR)
                    nc.vector.reciprocal_approx_fast(out=rbc, in_=dp)

                def av_step(co):
                    def go():
                        op = ps_big.tile([128, HW], F32, tag="big", name=f"av_{s}_{co}")
                        for mj in range(MJ):
                            lhsT = v8[:, 2 * mj:2 * mj + 2, co * 128:(co + 1) * 128]
                            for nh in range(NH):
                                nc.tensor.matmul(
                                    op[:, nh * 512:(nh + 1) * 512], lhsT=lhsT,
                                    rhs=pT[:, 2 * mj:2 * mj + 2, nh * 512:(nh + 1) * 512],
                                    start=(mj == 0), stop=(mj == MJ - 1), perf_mode=DR)
                        nc.vector.tensor_mul(o8[:, co, :], op, rbc)
                    return go
                return [den_step] + [av_step(co) for co in range(KO)]

            def proj_steps(s, o8):
                """proj through mm-pool [128,512] halves: finer pipelining
                at the kernel tail and leaves the big pool to the scores."""
                def p_step(co, nh):
                    def go():
                        nsl = slice(nh * 512, (nh + 1) * 512)
                        pp = ps_mm.tile([128, 512], F32, tag="mm",
                                        name=f"pp_{s}_{co}_{nh}")
                        for kj in range(KJ):
                            nc.tensor.matmul(
                                pp,
                                lhsT=wp_sb[:, 2 * kj:2 * kj + 2, co * 128:(co + 1) * 128],
                                rhs=o8[:, 2 * kj:2 * kj + 2, nsl],
                                start=(kj == 0), stop=(kj == KJ - 1), perf_mode=DR)
                        # y = pp/256 + (x+pb): the bias rides the host-
                        # precomputed bf16 residual, so the epilogue is one
                        # DVE op and ACT stays out of the tail entirely
                        y_sb = yp.tile([128, 512], BF16, tag="y", name=f"y_{s}_{co}_{nh}")
                        nc.vector.scalar_tensor_tensor(
                            out=y_sb, in0=pp, scalar=i256_sb,
                            in1=xpb_sbs[s][co][:, nsl], op0=OP.mult, op1=OP.add)
                        eng = nc.sync if nh == 0 else nc.gpsimd
                        eng.dma_start(
                            out=y_h[s][co * 128:(co + 1) * 128, nsl], in_=y_sb)
                    return go
                return [p_step(co, nh) for co in range(KO) for nh in range(NH)]

            def interleave(a, b, ratio):
                ai = bi = 0
                while ai < len(a) or bi < len(b):
                    for _ in range(ratio):
                        if ai < len(a):
                            a[ai]()
                            ai += 1
                    if bi < len(b):
                        b[bi]()
                        bi += 1

            o8s = [work.tile([128, KO, HW], F8, tag="o", name=f"o_{s}")
                   for s in range(SPC)]

            # ---------------- schedule ----------------
            st0 = emit_stats(0)
            scl0, off0 = emit_scloff(0, st0)
            # second junk burst bridges the PE to hn0-readiness so the HAM
            # clock-gate stays warm into the first real matmul phase
            emit_warmup(N_WARM2)
            hn0 = emit_hn(0, scl0, off0)
            ts0, t80 = t_steps(0, hn0)
            for f in ts0:
                f()
            emit_warmup(N_WARM3)
            vs0, v80, pT0 = vsc_steps(0, hn0, t80)
            for f in vs0:
                f()
            # sample 1's stats are pushed into the denav0 window with a
            # scheduling override: ready-first relaxed execution would
            # otherwise let bn1 starve sample 0's critical scl/off/hn chain
            # on the DVE as soon as x1 lands
            with tc.tile_wait_until(0.018):
                st1 = emit_stats(1)
                scl1, off1 = emit_scloff(1, st1)
            hn1 = emit_hn(1, scl1, off1)
            # sample 0's den/attn@v interleave with sample 1's t matmuls;
            # a t1 step leads so the PE queue isn't head-of-line blocked on
            # den0's wait for the last exp0
            da0 = denav_steps(0, v80, pT0, o8s[0])
            ts1, t81 = t_steps(1, hn1)
            interleave(ts1, da0, 1)
            # sample 1's fused v/scores (ACT-paced) overlap sample 0's proj
            vs1, v81, pT1 = vsc_steps(1, hn1, t81)
            pr0 = proj_steps(0, o8s[0])
            interleave(vs1, pr0, 1)
            da1 = denav_steps(1, v81, pT1, o8s[1])
            for f in da1:
                f()
            for f in proj_steps(1, o8s[1]):
                f()

    n = dedup_ldweights(nc)
    nc.compile()
    return nc


_NC_CACHE: dict = {}


def _get_nc() -> bass.Bass:
    if "nc" not in _NC_CACHE:
        _NC_CACHE["nc"] = build()
    return _NC_CACHE["nc"]


def make_in_maps(x, gamma, beta, qkv_w, qkv_b, proj_w, proj_b):
    import ml_dtypes
    f8 = np.dtype(ml_dtypes.float8_e4m3)
    f32 = np.float32
    x = np.ascontiguousarray(np.asarray(x, dtype=f32)).reshape(B, C, HW)
    qkv_w = np.asarray(qkv_w, dtype=np.float64)
    qkv_b = np.asarray(qkv_b, dtype=np.float64)
    proj_w = np.asarray(proj_w, dtype=np.float64)
    proj_b = np.asarray(proj_b, dtype=np.float64)
    wq, wk, wv = qkv_w[0:C], qkv_w[C:2 * C], qkv_w[2 * C:3 * C]
    u16 = np.zeros((C, 16), dtype=np.float64)
    u16[:, 0] = 16.0 * (wk.T @ qkv_b[0:C])
    kron = np.kron(np.eye(8, dtype=f32), np.ones((16, 16), dtype=f32))
    shared = {
        "m16": np.ascontiguousarray(16.0 * (wq.T @ wk)).astype(f8),
        "wv16": np.ascontiguousarray(16.0 * wv.T).astype(f8),
        "wp16": np.ascontiguousarray(16.0 * proj_w.T).astype(f8),
        "u16": u16.astype(f8),
        "pb": (proj_w @ qkv_b[2 * C:3 * C] + proj_b).astype(f32),
        "gam": np.ascontiguousarray(np.asarray(gamma, dtype=f32)),
        "bet": np.ascontiguousarray(np.asarray(beta, dtype=f32)),
        "gs": (kron * f32(1.0 / 16.0)).astype(np.dtype(ml_dtypes.bfloat16)),
        "on16": np.full((128, 256), 1.0 / 16.0, dtype=f8),
    }
    bf = np.dtype(ml_dtypes.bfloat16)
    xb = x.astype(bf)
    xpb = (x + shared["pb"][None, :, None]).astype(bf)
    return [dict(shared, xb=np.ascontiguousarray(xb[i * SPC:(i + 1) * SPC]),
                 xpb=np.ascontiguousarray(xpb[i * SPC:(i + 1) * SPC]))
            for i in range(NCORES)]


def run(x, gamma, beta, qkv_w, qkv_b, proj_w, proj_b, trace=False):
    in_maps = make_in_maps(x, gamma, beta, qkv_w, qkv_b, proj_w, proj_b)
    nc = _get_nc()
    res = run_bass_kernel_spmd(nc, in_maps, list(range(NCORES)), trace=trace)
    y = np.concatenate([res.results[i]["y"] for i in range(NCORES)], axis=0)
    return y.reshape(B, C, 32, 32).astype(np.float32), res


def kernel(**inputs) -> np.ndarray:
    y, _ = run(**inputs)
    return y


# revision 6
# speedup vs baseline: 1.2077x; 1.2077x over previous
"""Trainium2 Bass kernel for nn_AttentionBlock — fp8e4 DoubleRow edition.

Math (equivalent to the reference up to fp rounding):
  - GroupNorm folded to per-channel scale/offset. Per-channel (mean, E[x^2])
    via DVE bn_stats/bn_aggr (one pass, no ACT work); group reduce via a
    bf16 block-diagonal matmul; rstd = 1/sqrt(var+eps) by two DVE Newton
    steps from r0=1 (var is 1 +- ~6% here). ACT thus only ever runs
    {Identity, Exp}: one activation-table load for the whole kernel.
  - Scores computed as hn^T (wq^T wk) hn with M = wq^T wk precomputed on
    the host: q and k never materialize. The q-bias term becomes a per-m
    additive bias u^T hn (u = wk^T bq), computed by tiny N=1 matmuls that
    share the v-matmul's LDWEIGHTS, and fed to Exp's per-partition bias.
  - k bias dropped (constant along softmax axis); v bias folded into the
    proj bias pb = proj_w @ bv + proj_b, which rides the host-precomputed
    bf16 residual x+pb, so the epilogue is one DVE op per tile.
  - The x fed to stats/hn and the residual are bf16 (halves the
    HBM-contended startup DMA and the output traffic; rel err 2.5e-3 vs
    the 2e-2 gate; the attention path is fp8 anyway).
  - Softmax denominator: ones(1/16)-matmul accumulated over pT on the PE
    -> rbc = 16/den; o lands 16x scaled, proj weights are 16x scaled, the
    epilogue applies 1/256. All fp8 tensors sit in e4m3's normal range.
  - All big matmuls are fp8e4 DoubleRow: 2 weights/PE cell, K=256 per
    instruction, 2x bf16 FLOP rate.

Scheduling notes:
  - v, ub and both score halves for one token block share one LDWEIGHTS
    (lhsT = the same hn chunk pair); a pre-compile pass drops the
    redundant InstLdweights that bass emits per matmul (walrus does not
    dedup identical consecutive weight loads itself).
  - Softmax-denominator + attn@v of sample s interleave with sample s+1's
    t/v/scores matmuls; sample 1's scores interleave with sample 0's proj.
  - Junk DoubleRow matmuls with no DMA deps run at startup so the PE HAM
    clock-gate is warm when the real stream begins.

Measured numpy simulation of this exact quantization layout: rel err
7.7e-4 vs the fp32 reference (gate is 2e-2; the output is dominated by
the fp32 residual x, diluting attention-path noise ~30x).
"""

import math
import numpy as np

import concourse.bass as bass
import concourse.bacc as bacc
import concourse.tile as tile
from concourse import bass_isa, mybir
from concourse.bass_utils import run_bass_kernel_spmd

F32 = mybir.dt.float32
F8 = mybir.dt.float8e4
AF = mybir.ActivationFunctionType
OP = mybir.AluOpType
AX = mybir.AxisListType
DR = mybir.MatmulPerfMode.DoubleRow

B = 16
C = 512
HW = 1024
NCORES = 8
SPC = B // NCORES          # samples per core
KO = C // 128              # channel chunks of 128
KJ = KO // 2               # DoubleRow channel-chunk pairs
MI = HW // 128             # token chunks of 128
MJ = MI // 2               # DoubleRow token-chunk pairs
NH = HW // 512             # 512-wide column halves
EPS = 1e-5
SM_SCALE = 1.0 / math.sqrt(C)
N_WARM = 14
N_WARM2 = 12
N_WARM3 = 12
BF16 = mybir.dt.bfloat16


def dedup_ldweights(nc) -> int:
    """Drop InstLdweights that reload the exact weights already resident.

    bass emits one InstLdweights per matmul; for runs of matmuls sharing
    the same stationary operand only the first load is needed. Safe iff
    between the kept load and the candidate there are only PE matmuls
    (any other PE instruction — waits, drains — may order a write to the
    weights region, so it resets the window) and the candidate introduces
    no dependency edges beyond the kept load's.
    """
    removed = 0
    for blk in nc.main_func.blocks:
        insts = blk.instructions
        keep = []
        last_sig = None
        last_deps: tuple[set, set] | None = None
        between: set[str] = set()
        for i in insts:
            if isinstance(i, mybir.InstLdweights):
                sig = (str(i.ins[0]), str(getattr(i, "perf_mode", None)),
                       str(getattr(i, "tile_position", None)))
                sd = set(i.sync_dependency_names())
                nd = set(i.nosync_dependency_names())
                if (last_sig == sig and last_deps is not None
                        and sd <= last_deps[0]
                        and nd <= (last_deps[1] | between)):
                    removed += 1
                    continue
                last_sig = sig
                last_deps = (sd, nd)
                between = set()
            elif isinstance(i, mybir.InstMatmult):
                between.add(i.name)
            elif i.engine == mybir.EngineType.PE:
                last_sig = None
                last_deps = None
            keep.append(i)
        if removed:
            del insts[:]
            insts.extend(keep)
    return removed


def build() -> bass.Bass:
    nc = bacc.Bacc()

    xb_h = nc.declare_dram_parameter("xb", [SPC, C, HW], BF16, isOutput=False)
    xpb_h = nc.declare_dram_parameter("xpb", [SPC, C, HW], BF16, isOutput=False)
    m_h = nc.declare_dram_parameter("m16", [C, C], F8, isOutput=False)
    wv_h = nc.declare_dram_parameter("wv16", [C, C], F8, isOutput=False)
    wp_h = nc.declare_dram_parameter("wp16", [C, C], F8, isOutput=False)
    u_h = nc.declare_dram_parameter("u16", [C, 16], F8, isOutput=False)
    pb_h = nc.declare_dram_parameter("pb", [C], F32, isOutput=False)
    gam_h = nc.declare_dram_parameter("gam", [C], F32, isOutput=False)
    bet_h = nc.declare_dram_parameter("bet", [C], F32, isOutput=False)
    gs_h = nc.declare_dram_parameter("gs", [128, 128], mybir.dt.bfloat16, isOutput=False)
    on_h = nc.declare_dram_parameter("on16", [128, 256], F8, isOutput=False)
    y_h = nc.declare_dram_parameter("y", [SPC, C, HW], BF16, isOutput=True)

    with tile.TileContext(nc) as tc:
        with (
            tc.tile_pool(name="const", bufs=1) as const,
            tc.tile_pool(name="xp", bufs=2) as xp,
            tc.tile_pool(name="work", bufs=1) as work,
            tc.tile_pool(name="hnp", bufs=2) as hnp,
            tc.tile_pool(name="small", bufs=2) as small,
            tc.tile_pool(name="yp", bufs=3) as yp,
            tc.tile_pool(name="ps_mm", bufs=3, space="PSUM") as ps_mm,
            tc.tile_pool(name="ps_big", bufs=2, space="PSUM") as ps_big,
            tc.tile_pool(name="ps_ub", bufs=1, space="PSUM") as ps_ub,
        ):
            # ---- PE warmup (HAM un-throttle). Junk MMs go to the ub
            # pool's bank: its tiles have no readers, so the junk stream
            # never waits on evacuations and can fill any dependency gap.
            junkW = const.tile([128, 2, 128], F8, tag="junkW")
            nc.vector.memset(junkW, 0.0)
            junkR = const.tile([128, 2, 512], F8, tag="junkR")
            nc.vector.memset(junkR, 0.0)
            warm_n = [0]

            def emit_warmup(n):
                for _ in range(n):
                    wps = ps_mm.tile([128, 512], F32, tag="mm",
                                     name=f"warm_{warm_n[0]}")
                    warm_n[0] += 1
                    nc.tensor.matmul(wps, lhsT=junkW, rhs=junkR,
                                     start=True, stop=True, perf_mode=DR,
                                     skip_group_check=True)

            emit_warmup(N_WARM)

            # ---- input DMAs. The stats/hn path reads a bf16 copy of x
            # (half the bytes of the HBM-contended startup burst); the fp32
            # x for the residual streams later, it is first read ~40us in.
            xb_sbs = [[xp.tile([128, HW], BF16, tag=f"xb{ko}", name=f"xb_sb_{s}_{ko}")
                       for ko in range(KO)] for s in range(SPC)]
            # half-chunk transfers so bn_stats can chase the DMA stream;
            # halves split across two DGE queues (sync + idle gpsimd)
            for ko in range(KO):
                for h in range(2):
                    eng = nc.sync if h == 0 else nc.gpsimd
                    eng.dma_start(
                        out=xb_sbs[0][ko][:, h * 512:(h + 1) * 512],
                        in_=xb_h[0][ko * 128:(ko + 1) * 128, h * 512:(h + 1) * 512])
            gs_sb = const.tile([128, 128], mybir.dt.bfloat16, tag="gs")
            nc.sync.dma_start(out=gs_sb, in_=gs_h[:])
            m_sb = const.tile([128, KO, C], F8, tag="m16")
            nc.sync.dma_start(out=m_sb, in_=m_h[:].rearrange("(ki p) n -> p ki n", p=128))
            wv_sb = const.tile([128, KO, C], F8, tag="wv")
            nc.sync.dma_start(out=wv_sb, in_=wv_h[:].rearrange("(ki p) n -> p ki n", p=128))
            u_sb = const.tile([128, KO, 16], F8, tag="u")
            nc.sync.dma_start(out=u_sb, in_=u_h[:].rearrange("(ki p) z -> p ki z", p=128))
            pb_sb = const.tile([128, KO], F32, tag="pb")
            nc.sync.dma_start(out=pb_sb, in_=pb_h[:].rearrange("(ko p) -> p ko", p=128))
            gam_sb = const.tile([128, KO], F32, tag="gam")
            nc.sync.dma_start(out=gam_sb, in_=gam_h[:].rearrange("(ko p) -> p ko", p=128))
            bet_sb = const.tile([128, KO], F32, tag="bet")
            nc.sync.dma_start(out=bet_sb, in_=bet_h[:].rearrange("(ko p) -> p ko", p=128))
            for ko in range(KO):
                nc.sync.dma_start(out=xb_sbs[1][ko],
                                  in_=xb_h[1][ko * 128:(ko + 1) * 128, :])
            on_sb = const.tile([128, 2, 128], F8, tag="on")
            nc.sync.dma_start(out=on_sb, in_=on_h[:].rearrange("p (t k) -> p t k", t=2))
            wp_sb = const.tile([128, KO, C], F8, tag="wp")
            nc.gpsimd.dma_start(out=wp_sb, in_=wp_h[:].rearrange("(ki p) n -> p ki n", p=128))
            # residual-with-bias copies, first read by the proj epilogues
            xpb_sbs = [[xp.tile([128, HW], BF16, tag=f"xpb{ko}", name=f"xpb_sb_{s}_{ko}")
                        for ko in range(KO)] for s in range(SPC)]
            for s in range(SPC):
                for ko in range(KO):
                    nc.sync.dma_start(out=xpb_sbs[s][ko],
                                      in_=xpb_h[s][ko * 128:(ko + 1) * 128, :])

            eps_sb = const.tile([128, 1], F32, tag="eps")
            nc.vector.memset(eps_sb, EPS)
            i16_sb = const.tile([128, 1], F32, tag="i16")
            nc.vector.memset(i16_sb, 1.0 / 16.0)
            smsc_sb = const.tile([128, 1], F32, tag="smsc")
            nc.vector.memset(smsc_sb, SM_SCALE / 16.0)
            i256_sb = const.tile([128, 1], F32, tag="i256")
            nc.vector.memset(i256_sb, 1.0 / 256.0)
            zero_sb = const.tile([128, 1], F32, tag="zero")
            nc.vector.memset(zero_sb, 0.0)
            junk_f = const.tile([128, HW], F32, tag="junkF")
            m05_sb = const.tile([128, 1], F32, tag="m05")
            nc.vector.memset(m05_sb, -0.5)
            c15_sb = const.tile([128, 1], F32, tag="c15")
            nc.vector.memset(c15_sb, 1.5)

            def emit_stats(s, act_chunks=()):
                """(mean, E[x^2]) per channel: DVE bn_stats one-pass; the
                chunks in act_chunks instead use ACT accumulation with the
                scale trick (Identity(x/1024) sums to the mean, Square(x/32)
                sums to E[x^2]) to offload the DVE where it is congested.
                st in bf16 so the group matmul runs single-pass."""
                bn = small.tile([128, KO, 2, 6], F32, tag="bn", name=f"bn_{s}")
                st = small.tile([128, KO, 2], mybir.dt.bfloat16, tag="st",
                                name=f"st_{s}")
                for ko in range(KO):
                    if ko in act_chunks:
                        # ACT's accumulator is fp32 internally; only the
                        # final store rounds to bf16
                        with nc.allow_low_precision(reason="fp32 accum, bf16 store"):
                            nc.scalar.activation(out=junk_f, in_=xb_sbs[s][ko],
                                                 func=AF.Identity, bias=zero_sb,
                                                 scale=1.0 / 1024.0,
                                                 accum_out=st[:, ko, 0:1])
                            nc.scalar.activation(out=junk_f, in_=xb_sbs[s][ko],
                                                 func=AF.Square, bias=zero_sb,
                                                 scale=1.0 / 32.0,
                                                 accum_out=st[:, ko, 1:2])
                        continue
                    for h in range(2):
                        nc.vector.bn_stats(out=bn[:, ko, h, :],
                                           in_=xb_sbs[s][ko][:, h * 512:(h + 1) * 512])
                    nc.vector.bn_aggr(out=st[:, ko, :], in_=bn[:, ko, :, :])
                # var -> E[x^2] fixup, only for the bn chunks
                msq = small.tile([128, KO], F32, tag="msq", name=f"msq_{s}")
                for ko in range(KO):
                    if ko in act_chunks:
                        continue
                    nc.vector.tensor_mul(msq[:, ko:ko + 1], st[:, ko, 0:1],
                                         st[:, ko, 0:1])
                    nc.vector.tensor_add(st[:, ko, 1:2], st[:, ko, 1:2],
                                         msq[:, ko:ko + 1])
                return st

            def emit_scloff(s, st):
                gpt = ps_mm.tile([128, 512], F32, tag="mm", name=f"gps_{s}")
                for ko in range(KO):
                    nc.tensor.matmul(gpt[:, 2 * ko:2 * ko + 2], lhsT=gs_sb,
                                     rhs=st[:, ko, :], start=True, stop=True)
                gps = gpt[:, 0:8].rearrange("p (ko t) -> p ko t", t=2)
                mean = small.tile([128, KO], F32, tag="mean", name=f"mean_{s}")
                nc.vector.tensor_copy(out=mean, in_=gps[:, :, 0])
                var = small.tile([128, KO], F32, tag="var", name=f"var_{s}")
                nc.vector.tensor_mul(var, mean, mean)
                nc.vector.tensor_sub(var, gps[:, :, 1], var)
                # rstd = 1/sqrt(var+eps) via two Newton steps from r0=1 on
                # DVE (group var over 16k unit-normal samples is 1 +- ~6%,
                # so convergence is ~1e-5): keeps Sqrt off ACT, which then
                # needs exactly one activation-table set for the kernel.
                vpe = small.tile([128, KO], F32, tag="vpe", name=f"vpe_{s}")
                nc.vector.tensor_scalar_add(out=vpe, in0=var, scalar1=eps_sb)
                rstd = small.tile([128, KO], F32, tag="rstd", name=f"rstd_{s}")
                nc.vector.tensor_scalar(out=rstd, in0=vpe, scalar1=m05_sb,
                                        scalar2=c15_sb, op0=OP.mult, op1=OP.add)
                rr = small.tile([128, KO], F32, tag="rr", name=f"rr_{s}")
                nc.vector.tensor_mul(rr, rstd, rstd)
                nc.vector.tensor_mul(rr, vpe, rr)
                nc.vector.tensor_scalar(out=rr, in0=rr, scalar1=m05_sb,
                                        scalar2=c15_sb, op0=OP.mult, op1=OP.add)
                nc.vector.tensor_mul(rstd, rstd, rr)
                scl = small.tile([128, KO], F32, tag="scl", name=f"scl_{s}")
                nc.vector.tensor_mul(scl, rstd, gam_sb)
                off = small.tile([128, KO], F32, tag="off", name=f"off_{s}")
                nc.vector.tensor_mul(off, mean, scl)
                nc.vector.tensor_sub(off, bet_sb, off)
                return scl, off

            def emit_hn(s, scl, off):
                hn = hnp.tile([128, KO, HW], F8, tag="hn", name=f"hn_{s}")
                for ko in range(KO):
                    if ko in (1, 2) or s == 1:
                        nc.scalar.activation(
                            out=hn[:, ko, :], in_=xb_sbs[s][ko],
                            func=AF.Identity, bias=off[:, ko:ko + 1],
                            scale=scl[:, ko:ko + 1])
                    else:
                        nc.vector.tensor_scalar(
                            out=hn[:, ko, :], in0=xb_sbs[s][ko],
                            scalar1=scl[:, ko:ko + 1], scalar2=off[:, ko:ko + 1],
                            op0=OP.mult, op1=OP.add)
                return hn

            def t_steps(s, hn):
                """t = M^T hn, channel-major fp8 (ACT evacuates)."""
                t8 = work.tile([128, KO, HW], F8, tag="t", name=f"t_{s}")

                def t_step(mo):
                    def go():
                        tp = [ps_mm.tile([128, 512], F32, tag="mm",
                                         name=f"t_{s}_{mo}_{nh}") for nh in range(NH)]
                        for kj in range(KJ):
                            for nh in range(NH):
                                nc.tensor.matmul(
                                    tp[nh],
                                    lhsT=m_sb[:, 2 * kj:2 * kj + 2, mo * 128:(mo + 1) * 128],
                                    rhs=hn[:, 2 * kj:2 * kj + 2, nh * 512:(nh + 1) * 512],
                                    start=(kj == 0), stop=(kj == KJ - 1), perf_mode=DR)
                        # evacuations: split ACT/DVE for sample 0 (its t
                        # phase is latency-critical); all-ACT for sample 1
                        # (hidden under denav0, where the DVE is the busy one)
                        nc.scalar.activation(
                            out=t8[:, mo, 0:512], in_=tp[0],
                            func=AF.Identity, bias=eps_sb, scale=1.0 / 16.0)
                        if s == 0:
                            nc.vector.tensor_scalar_mul(
                                out=t8[:, mo, 512:1024], in0=tp[1], scalar1=i16_sb)
                        else:
                            nc.scalar.activation(
                                out=t8[:, mo, 512:1024], in_=tp[1],
                                func=AF.Identity, bias=eps_sb, scale=1.0 / 16.0)
                    return go
                return [t_step(mo) for mo in range(KO)], t8

            def vsc_steps(s, hn, t8):
                """Fused v / ub / scores / exp per token block: all four
                matmul kinds share one LDWEIGHTS of the hn chunk pair."""
                v8 = work.tile([128, MI, C], F8, tag="v", name=f"v_{s}")
                pT = work.tile([128, MI, HW], F8, tag="pT", name=f"pT_{s}")
                ub_sb = small.tile([128, MI], F32, tag="ubs", name=f"ubs_{s}")

                def step(mi):
                    def go():
                        vp = ps_mm.tile([128, 512], F32, tag="mm", name=f"v_{s}_{mi}")
                        up = ps_ub.tile([128, 512], F32, tag="ub", name=f"ub_{s}_{mi}")
                        sp = ps_big.tile([128, HW], F32, tag="big", name=f"sc_{s}_{mi}")
                        for kj in range(KJ):
                            lhsT = hn[:, 2 * kj:2 * kj + 2, mi * 128:(mi + 1) * 128]
                            st, sp_ = (kj == 0), (kj == KJ - 1)
                            nc.tensor.matmul(vp, lhsT=lhsT,
                                             rhs=wv_sb[:, 2 * kj:2 * kj + 2, :],
                                             start=st, stop=sp_, perf_mode=DR)
                            nc.tensor.matmul(up[:, 0:1], lhsT=lhsT,
                                             rhs=u_sb[:, 2 * kj:2 * kj + 2, 0:1],
                                             start=st, stop=sp_, perf_mode=DR)
                            for nh in range(NH):
                                nc.tensor.matmul(
                                    sp[:, nh * 512:(nh + 1) * 512], lhsT=lhsT,
                                    rhs=t8[:, 2 * kj:2 * kj + 2, nh * 512:(nh + 1) * 512],
                                    start=st, stop=sp_, perf_mode=DR)
                        if mi % 2 == 0:
                            nc.vector.tensor_scalar_mul(out=v8[:, mi, :], in0=vp,
                                                        scalar1=i16_sb)
                        else:
                            nc.scalar.activation(out=v8[:, mi, :], in_=vp,
                                                 func=AF.Identity, bias=eps_sb,
                                                 scale=1.0 / 16.0)
                        nc.vector.tensor_scalar_mul(out=ub_sb[:, mi:mi + 1],
                                                    in0=up[:, 0:1], scalar1=smsc_sb)
                        nc.scalar.activation(out=pT[:, mi, :], in_=sp,
                                             func=AF.Exp, bias=ub_sb[:, mi:mi + 1],
                                             scale=SM_SCALE)
                    return go
                return [step(mi) for mi in range(MI)], v8, pT

            def denav_steps(s, v8, pT, o8):
                rbc = work.tile([128, HW], F32, tag="rbc", name=f"rbc_{s}")

                def den_step():
                    dp = ps_big.tile([128, HW], F32, tag="big", name=f"den_{s}")
                    for nh in range(NH):
                        for mj in range(MJ):
                            nc.tensor.matmul(
                                dp[:, nh * 512:(nh + 1) * 512], lhsT=on_sb,
                                rhs=pT[:, 2 * mj:2 * mj + 2, nh * 512:(nh + 1) * 512],
                                start=(mj == 0), stop=(mj == MJ - 1), perf_mode=DR)
                    nc.vector.reciprocal_approx_fast(out=rbc, in_=dp)

                def av_step(co):
                    def go():
                        op = ps_big.tile([128, HW], F32, tag="big", name=f"av_{s}_{co}")
                        for mj in range(MJ):
                            lhsT = v8[:, 2 * mj:2 * mj + 2, co * 128:(co + 1) * 128]
                            for nh in range(NH):
                                nc.tensor.matmul(
                                    op[:, nh * 512:(nh + 1) * 512], lhsT=lhsT,
                                    rhs=pT[:, 2 * mj:2 * mj + 2, nh * 512:(nh + 1) * 512],
                                    start=(mj == 0), stop=(mj == MJ - 1), perf_mode=DR)
                        nc.vector.tensor_mul(o8[:, co, :], op, rbc)
                    return go
                return [den_step] + [av_step(co) for co in range(KO)]

            def proj_steps(s, o8):
                """proj through mm-pool [128,512] halves: finer pipelining
                at the kernel tail and leaves the big pool to the scores."""
                def p_step(co, nh):
                    def go():
                        nsl = slice(nh * 512, (nh + 1) * 512)
                        pp = ps_mm.tile([128, 512], F32, tag="mm",
                                        name=f"pp_{s}_{co}_{nh}")
                        for kj in range(KJ):
                            nc.tensor.matmul(
                                pp,
                                lhsT=wp_sb[:, 2 * kj:2 * kj + 2, co * 128:(co + 1) * 128],
                                rhs=o8[:, 2 * kj:2 * kj + 2, nsl],
                                start=(kj == 0), stop=(kj == KJ - 1), perf_mode=DR)
                        # y = pp/256 + (x+pb): the bias rides the host-
                        # precomputed bf16 residual, so the epilogue is one
                        # DVE op and ACT stays out of the tail entirely
                        y_sb = yp.tile([128, 512], BF16, tag="y", name=f"y_{s}_{co}_{nh}")
                        nc.vector.scalar_tensor_tensor(
                            out=y_sb, in0=pp, scalar=i256_sb,
                            in1=xpb_sbs[s][co][:, nsl], op0=OP.mult, op1=OP.add)
                        eng = nc.sync if nh == 0 else nc.gpsimd
                        eng.dma_start(
                            out=y_h[s][co * 128:(co + 1) * 128, nsl], in_=y_sb)
                    return go
                return [p_step(co, nh) for co in range(KO) for nh in range(NH)]

            def interleave(a, b, ratio):
                ai = bi = 0
                while ai < len(a) or bi < len(b):
                    for _ in range(ratio):
                        if ai < len(a):
                            a[ai]()
                            ai += 1
                    if bi < len(b):
                        b[bi]()
                        bi += 1

            o8s = [work.tile([128, KO, HW], F8, tag="o", name=f"o_{s}")
                   for s in range(SPC)]

            # ---------------- schedule ----------------
            st0 = emit_stats(0)
            scl0, off0 = emit_scloff(0, st0)
            # second junk burst bridges the PE to hn0-readiness so the HAM
            # clock-gate stays warm into the first real matmul phase
            emit_warmup(N_WARM2)
            hn0 = emit_hn(0, scl0, off0)
            ts0, t80 = t_steps(0, hn0)
            for f in ts0:
                f()
            emit_warmup(N_WARM3)
            vs0, v80, pT0 = vsc_steps(0, hn0, t80)
            for f in vs0:
                f()
            # sample 1's stats are pushed into the denav0 window with a
            # scheduling override: ready-first relaxed execution would
            # otherwise let bn1 starve sample 0's critical scl/off/hn chain
            # on the DVE as soon as x1 lands
            with tc.tile_wait_until(0.018):
                st1 = emit_stats(1)
                scl1, off1 = emit_scloff(1, st1)
            hn1 = emit_hn(1, scl1, off1)
            # sample 0's den/attn@v interleave with sample 1's t matmuls;
            # a t1 step leads so the PE queue isn't head-of-line blocked on
            # den0's wait for the last exp0
            da0 = denav_steps(0, v80, pT0, o8s[0])
            ts1, t81 = t_steps(1, hn1)
            interleave(ts1, da0, 1)
            # sample 1's fused v/scores (ACT-paced) overlap sample 0's proj
            vs1, v81, pT1 = vsc_steps(1, hn1, t81)
            pr0 = proj_steps(0, o8s[0])
            interleave(vs1, pr0, 1)
            da1 = denav_steps(1, v81, pT1, o8s[1])
            for f in da1:
                f()
            for f in proj_steps(1, o8s[1]):
                f()

    n = dedup_ldweights(nc)
    nc.compile()
    return nc


_NC_CACHE: dict = {}


def _get_nc() -> bass.Bass:
    if "nc" not in _NC_CACHE:
        _NC_CACHE["nc"] = build()
    return _NC_CACHE["nc"]


def make_in_maps(x, gamma, beta, qkv_w, qkv_b, proj_w, proj_b):
    import ml_dtypes
    f8 = np.dtype(ml_dtypes.float8_e4m3)
    f32 = np.float32
    x = np.ascontiguousarray(np.asarray(x, dtype=f32)).reshape(B, C, HW)
    qkv_w = np.asarray(qkv_w, dtype=np.float64)
    qkv_b = np.asarray(qkv_b, dtype=np.float64)
    proj_w = np.asarray(proj_w, dtype=np.float64)
    proj_b = np.asarray(proj_b, dtype=np.float64)
    wq, wk, wv = qkv_w[0:C], qkv_w[C:2 * C], qkv_w[2 * C:3 * C]
    u16 = np.zeros((C, 16), dtype=np.float64)
    u16[:, 0] = 16.0 * (wk.T @ qkv_b[0:C])
    kron = np.kron(np.eye(8, dtype=f32), np.ones((16, 16), dtype=f32))
    shared = {
        "m16": np.ascontiguousarray(16.0 * (wq.T @ wk)).astype(f8),
        "wv16": np.ascontiguousarray(16.0 * wv.T).astype(f8),
        "wp16": np.ascontiguousarray(16.0 * proj_w.T).astype(f8),
        "u16": u16.astype(f8),
        "pb": (proj_w @ qkv_b[2 * C:3 * C] + proj_b).astype(f32),
        "gam": np.ascontiguousarray(np.asarray(gamma, dtype=f32)),
        "bet": np.ascontiguousarray(np.asarray(beta, dtype=f32)),
        "gs": (kron * f32(1.0 / 16.0)).astype(np.dtype(ml_dtypes.bfloat16)),
        "on16": np.full((128, 256), 1.0 / 16.0, dtype=f8),
    }
    bf = np.dtype(ml_dtypes.bfloat16)
    xb = x.astype(bf)
    xpb = (x + shared["pb"][None, :, None]).astype(bf)
    return [dict(shared, xb=np.ascontiguousarray(xb[i * SPC:(i + 1) * SPC]),
                 xpb=np.ascontiguousarray(xpb[i * SPC:(i + 1) * SPC]))
            for i in range(NCORES)]


def run(x, gamma, beta, qkv_w, qkv_b, proj_w, proj_b, trace=False):
    in_maps = make_in_maps(x, gamma, beta, qkv_w, qkv_b, proj_w, proj_b)
    nc = _get_nc()
    res = run_bass_kernel_spmd(nc, in_maps, list(range(NCORES)), trace=trace)
    y = np.concatenate([res.results[i]["y"] for i in range(NCORES)], axis=0)
    return y.reshape(B, C, 32, 32).astype(np.float32), res


def kernel(**inputs) -> np.ndarray:
    y, _ = run(**inputs)
    return y
